# revision 1
# baseline (speedup 1.0000x reference)
"""GAT layer (nn_GATLayer) Trainium2 Bass kernel.

Reference math:
    NF = x @ W.T + b                     # [N, 256] -> heads [N, 8, 32]
    lp[i,h] = sum_d NF[i,h,d] * a[h,d];  lc[j,h] = sum_d NF[j,h,d] * a[h,32+d]
    logits[i,j,h] = leaky_relu(lp+lc, 0.2) masked to 0 where adj==0
    out[i,h,:] = softmax_j(logits) @ NF[:,h,:]

Identities used (adj in {0,1}):
    exp(leaky_relu(z, .2)) = exp(.2 z) * max(exp(.8 z), 1)
    em[i,j,h] = exp(masked logits) = 1 - adj + adj*A2[i]*B2[j]*max(A8[i]*B8[j], 1)
      where A8=exp(.8 lp), B8=exp(.8 lc), A2=exp(.2 lp), B2=exp(.2 lc)
    u'[j,i,h] = adj[i,j] * max(A8[i]*B8[j], 1)        # ONE masked stream per head
    num[i,h,c] = S[h,c] - M3[i,c] + A2[i,h] * (u' @ (B2*NF)_h)[i,c]
    Z[i,h]     = N - deg[i]      + A2[i,h] * (u' @ B2_h)[i]
    out = num / Z
    with S = colsum(NF), M3 = adj @ NF, deg = adj @ 1.

Per core (rows sharded, 512 each): j-loop over 32 chunks of 128, two rounds
to fit PSUM banks: round A = shared(M3) psums + heads 0-5, round B = deg +
heads 6-7. u' built as: TS (a8m = (A8rep * B8[j]) max 1) + TT (a8m * adjT),
bf16, with some head-tiles offloaded to ACT (2-op max via relu) and TT
slices to GPSIMD for engine balance.
"""

import numpy as np
import ml_dtypes

import concourse.bass as bass
import concourse.bacc as bacc
import concourse.tile as tile
from concourse import mybir
from concourse.bass_utils import run_bass_kernel_spmd
from concourse.masks import make_identity

N_CORES = 8
N = 4096
IN_FEAT = 256
OUT_FEAT = 256
H = 8
D = 32
R = N // N_CORES          # rows (parents) per core = 512
JC = N // 128             # j-chunks of 128 = 32
KA = IN_FEAT + 1          # augmented contraction (bias row) = 257
WCOLS = D + 1             # per-head weight cols (B2NF slice + B2 col) = 33

FP = mybir.dt.float32
BF = mybir.dt.bfloat16

ROUND_A = [0, 1, 2, 3, 4, 7]
ROUND_B = [5, 6]
# engine assignment for the a8m op per head: 'dve' (TS) or 'act' (2 ACT ops)
A8M_ENGINE = {0: 'dve', 1: 'dve', 2: 'dve', 3: 'dve', 4: 'dve', 5: 'dve',
              6: 'dve', 7: 'act'}
# TT (mask multiply) engine per head
TT_ENGINE = {0: 'dve', 1: 'dve', 2: 'dve', 3: 'dve', 4: 'gp', 5: 'gp',
             6: 'dve', 7: 'gp'}


def build_program():
    nc = bacc.Bacc("TRN2", target_bir_lowering=False, debug=False,
                   num_devices=N_CORES)

    xTa = nc.dram_tensor("xTa", [KA, N], FP, kind="ExternalInput").ap()
    xTrows = nc.dram_tensor("xTrows", [KA, R], FP, kind="ExternalInput").ap()
    wTa = nc.dram_tensor("wTa", [KA, OUT_FEAT + H], FP, kind="ExternalInput").ap()
    WAc = nc.dram_tensor("WAc", [KA, H], FP, kind="ExternalInput").ap()
    WAp = nc.dram_tensor("WAp", [KA, H], FP, kind="ExternalInput").ap()
    adjT = nc.dram_tensor("adjT", [N, R], BF, kind="ExternalInput").ap()
    sel8_in = nc.dram_tensor("sel8", [H, H * 128], FP, kind="ExternalInput").ap()
    sel32_in = nc.dram_tensor("sel32", [H, 256], FP, kind="ExternalInput").ap()
    outT = nc.dram_tensor("outT", [OUT_FEAT, R], FP, kind="ExternalOutput").ap()

    with tile.TileContext(nc) as tc:
        from contextlib import ExitStack
        with ExitStack() as top:
            consts = top.enter_context(tc.tile_pool(name="consts", bufs=1))
            persist = top.enter_context(tc.tile_pool(name="persist", bufs=1))

            ident = consts.tile([128, 128], FP)
            make_identity(nc, ident[:])
            ones_col = consts.tile([128, 1], BF)
            nc.vector.memset(ones_col[:], 1.0)
            sel8 = consts.tile([H, H * 128], FP)
            nc.sync.dma_start(out=sel8[:], in_=sel8_in[:])
            sel32 = consts.tile([H, 256], FP)
            nc.sync.dma_start(out=sel32[:], in_=sel32_in[:])
            negone = consts.tile([128, 1], FP)
            nc.vector.memset(negone[:], -1.0)

            wk = consts.tile([128, 2, OUT_FEAT + H], FP)
            nc.sync.dma_start(out=wk[:, 0, :], in_=wTa[0:128, :])
            nc.sync.dma_start(out=wk[:, 1, :], in_=wTa[128:256, :])
            wk2 = consts.tile([1, OUT_FEAT + H], FP)
            nc.sync.dma_start(out=wk2[:], in_=wTa[256:257, :])
            wap = consts.tile([128, 2, H], FP)
            nc.sync.dma_start(out=wap[:, 0, :], in_=WAp[0:128, :])
            nc.sync.dma_start(out=wap[:, 1, :], in_=WAp[128:256, :])
            wap2 = consts.tile([1, H], FP)
            nc.sync.dma_start(out=wap2[:], in_=WAp[256:257, :])

            # Persistent SBUF
            aggW = persist.tile([128, JC, H * WCOLS], BF)   # [B2NF_h | B2_h] x8
            shW = persist.tile([128, JC, OUT_FEAT + 1], BF)  # [NF | ones]
            lcn = persist.tile([128, JC, H], FP)
            b8c = persist.tile([128, JC, H], FP)             # exp(.8 lc)
            lpT = persist.tile([H, R], FP)
            a8rep = persist.tile([128, H, R], BF)            # exp(.8 lp) bcast
            a2rep = persist.tile([128, H, R], FP)            # exp(.2 lp) bcast
            scol = persist.tile([128, 2], FP)
            numT = persist.tile([128, 2, R], FP)
            outTs = persist.tile([128, 2, R], FP)
            zAll32 = persist.tile([32, R], FP)
            rzAll = persist.tile([32, R], FP)
            m3s = persist.tile([128, 2, R], FP)
            zrow6 = persist.tile([1, H, R], FP)
            degs = persist.tile([1, R], FP)
            nc.vector.memset(zAll32[:], 1.0)

            aggW_v = aggW.rearrange("p j (h w) -> p j h w", w=WCOLS)
            nc.vector.memset(shW[:, :, OUT_FEAT:OUT_FEAT + 1], 1.0)

            # ---- Phase 0: NF, lc, lp, exp factors, S ----
            with ExitStack() as ph0:
                xw = ph0.enter_context(tc.tile_pool(name="xw", bufs=3))
                ps0 = ph0.enter_context(
                    tc.tile_pool(name="ps0", bufs=4, space="PSUM"))
                for nb in range(JC):
                    xk = xw.tile([128, 2, 128], FP)
                    nc.sync.dma_start(out=xk[:, 0, :],
                                      in_=xTa[0:128, nb * 128:(nb + 1) * 128])
                    nc.sync.dma_start(out=xk[:, 1, :],
                                      in_=xTa[128:256, nb * 128:(nb + 1) * 128])
                    xk2 = xw.tile([1, 128], FP)
                    nc.sync.dma_start(out=xk2[:],
                                      in_=xTa[256:257, nb * 128:(nb + 1) * 128])
                    pnf = ps0.tile([128, OUT_FEAT + H], FP, space="PSUM",
                                   tag="ps0")
                    nc.tensor.matmul(pnf[:], xk[:, 0, :], wk[:, 0, :],
                                     start=True, stop=False)
                    nc.tensor.matmul(pnf[:], xk[:, 1, :], wk[:, 1, :],
                                     start=False, stop=False)
                    nc.tensor.matmul(pnf[:], xk2[:], wk2[:],
                                     start=False, stop=True)
                    # NF (bf16) into shared weights; lc tail into lcn
                    nc.scalar.copy(shW[:, nb, 0:OUT_FEAT], pnf[:, 0:OUT_FEAT])
                    nc.scalar.copy(lcn[:, nb, :], pnf[:, OUT_FEAT:OUT_FEAT + H])

                # exp factors of lc: B8 = exp(.8 lc) (scalar APs), B2 = exp(.2 lc)
                lcn_f = lcn.rearrange("p j h -> p (j h)")
                b8c_f = b8c.rearrange("p j h -> p (j h)")
                nc.scalar.activation(b8c_f, lcn_f,
                                     mybir.ActivationFunctionType.Exp,
                                     bias=0.0, scale=0.8)
                b2tmp = xw.tile([128, JC, H], FP, name="b2tmp")
                nc.scalar.activation(b2tmp.rearrange("p j h -> p (j h)"), lcn_f,
                                     mybir.ActivationFunctionType.Exp,
                                     bias=0.0, scale=0.2)
                # aggW: B2NF = NF * B2 (broadcast B2 over the 32 feature cols)
                for nb in range(JC):
                    b2b = b2tmp[:, nb, :]
                    b2bc = bass.AP(tensor=b2b.tensor, offset=b2b.offset,
                                   ap=[b2b.ap[0], b2b.ap[1], [0, D]])
                    nfv = shW[:, nb, 0:OUT_FEAT].rearrange(
                        "p (h d) -> p h d", d=D)
                    nc.gpsimd.tensor_mul(aggW_v[:, nb, :, 0:D], nfv, b2bc)
                    b2col = b2tmp[:, nb, :]
                    b2col3 = bass.AP(tensor=b2col.tensor, offset=b2col.offset,
                                     ap=[b2col.ap[0], b2col.ap[1], [0, 1]])
                    nc.gpsimd.tensor_copy(aggW_v[:, nb, :, D:D + 1], b2col3)

                # lp for this core's own rows
                for rb in range(R // 128):
                    xr = xw.tile([128, 2, 128], FP)
                    nc.sync.dma_start(out=xr[:, 0, :],
                                      in_=xTrows[0:128, rb * 128:(rb + 1) * 128])
                    nc.sync.dma_start(out=xr[:, 1, :],
                                      in_=xTrows[128:256, rb * 128:(rb + 1) * 128])
                    xr2 = xw.tile([1, 128], FP)
                    nc.sync.dma_start(
                        out=xr2[:], in_=xTrows[256:257, rb * 128:(rb + 1) * 128])
                    plp = ps0.tile([128, H], FP, space="PSUM", tag="ps0")
                    nc.tensor.matmul(plp[:], xr[:, 0, :], wap[:, 0, :],
                                     start=True, stop=False)
                    nc.tensor.matmul(plp[:], xr[:, 1, :], wap[:, 1, :],
                                     start=False, stop=False)
                    nc.tensor.matmul(plp[:], xr2[:], wap2[:],
                                     start=False, stop=True)
                    lps = xw.tile([128, H], FP)
                    nc.scalar.copy(lps[:], plp[:])
                    plpT = ps0.tile([H, 128], FP, space="PSUM", tag="ps0")
                    nc.tensor.transpose(plpT[:], lps[:], ident[:])
                    nc.scalar.copy(lpT[:, rb * 128:(rb + 1) * 128], plpT[:])

                # S = colsum(NF) from bf16 weights (matches aggregation dtype)
                psS = ps0.tile([1, OUT_FEAT], FP, space="PSUM", tag="ps0")
                for nb in range(JC):
                    nc.tensor.matmul(psS[:], ones_col[:], shW[:, nb, 0:OUT_FEAT],
                                     start=(nb == 0), stop=(nb == JC - 1))
                sS = xw.tile([1, OUT_FEAT], FP)
                nc.scalar.copy(sS[:], psS[:])
                for h in range(H):
                    pst = ps0.tile([D, 1], FP, space="PSUM", tag="ps0")
                    nc.tensor.transpose(
                        pst[:], sS[0:1, h * D:(h + 1) * D], ident[0:1, 0:1])
                    r0 = (h * D) % 128
                    nc.scalar.copy(scol[r0:r0 + D, h // 4:h // 4 + 1], pst[:])

                # A8/A2 row factors, broadcast across partitions
                a8T = xw.tile([H, R], FP)
                nc.scalar.activation(a8T[:], lpT[:],
                                     mybir.ActivationFunctionType.Exp,
                                     bias=0.0, scale=0.8)
                a2T = xw.tile([H, R], FP)
                nc.scalar.activation(a2T[:], lpT[:],
                                     mybir.ActivationFunctionType.Exp,
                                     bias=0.0, scale=0.2)
                for h in range(H):
                    pbr = ps0.tile([128, R], FP, space="PSUM", tag="psbig")
                    nc.tensor.matmul(pbr[:], sel8[:, h * 128:(h + 1) * 128],
                                     a8T[:], start=True, stop=True)
                    nc.vector.tensor_copy(a8rep[:, h, :], pbr[:])
                    pbr2 = ps0.tile([128, R], FP, space="PSUM", tag="psbig")
                    nc.tensor.matmul(pbr2[:], sel8[:, h * 128:(h + 1) * 128],
                                     a2T[:], start=True, stop=True)
                    nc.scalar.copy(a2rep[:, h, :], pbr2[:])

            # ---- Phase 1: main j-loop, two rounds ----
            def do_round(ph, heads, with_m3, with_deg):
                acc = ph.enter_context(
                    tc.tile_pool(name="acc", bufs=1, space="PSUM"))
                stream = ph.enter_context(tc.tile_pool(name="stream", bufs=3))
                work = ph.enter_context(tc.tile_pool(name="work", bufs=3))
                pacc = {h: acc.tile([WCOLS, R], FP, space="PSUM",
                                    name=f"pacc{h}") for h in heads}
                psh = None
                pdeg = None
                if with_m3:
                    psh = [acc.tile([128, R], FP, space="PSUM", name=f"psh{k}")
                           for k in range(2)]
                if with_deg:
                    pdeg = acc.tile([1, R], FP, space="PSUM", name="pdeg")
                nh = len(heads)
                for jc in range(JC):
                    at = stream.tile([128, R], BF, name="adjT_tile")
                    nc.sync.dma_start(out=at[:],
                                      in_=adjT[jc * 128:(jc + 1) * 128, :])
                    tb = work.tile([128, nh, R], BF, name="tb")
                    s1 = work.tile([128, nh, R], BF, name="s1")
                    for k, h in enumerate(heads):
                        if A8M_ENGINE[h] == 'act':
                            # relu(a8*B8 - 1) then +1 == max(a8*B8, 1)
                            rr = work.tile([128, R], BF, name="rr")
                            nc.scalar.activation(
                                rr[:], a8rep[:, h, :],
                                mybir.ActivationFunctionType.Relu,
                                bias=negone[:], scale=b8c[:, jc, h:h + 1])
                            nc.scalar.activation(
                                tb[:, k, :], rr[:],
                                mybir.ActivationFunctionType.Copy,
                                bias=1.0, scale=1.0)
                        else:
                            nc.vector.tensor_scalar(
                                tb[:, k, :], a8rep[:, h, :],
                                b8c[:, jc, h:h + 1], 1.0,
                                mybir.AluOpType.mult, mybir.AluOpType.max)
                    # mask multiply: fused across heads per engine
                    dve_ks = [k for k, h in enumerate(heads)
                              if TT_ENGINE[h] == 'dve']
                    gp_ks = [k for k, h in enumerate(heads)
                             if TT_ENGINE[h] == 'gp']
                    for eng, ks in ((nc.vector, dve_ks), (nc.gpsimd, gp_ks)):
                        for k0, k1 in _runs(ks):
                            cnt = k1 - k0
                            atb = bass.AP(tensor=at.tensor, offset=at.offset,
                                          ap=[at.ap[0], [0, cnt], at.ap[1]])
                            eng.tensor_mul(s1[:, k0:k1, :], tb[:, k0:k1, :],
                                           atb)
                    for k, h in enumerate(heads):
                        nc.tensor.matmul(
                            pacc[h][:],
                            aggW[:, jc, h * WCOLS:(h + 1) * WCOLS],
                            s1[:, k, :],
                            start=(jc == 0), stop=(jc == JC - 1))
                    if with_m3:
                        nc.tensor.matmul(psh[0][:], shW[:, jc, 0:128], at[:],
                                         start=(jc == 0), stop=(jc == JC - 1))
                        nc.tensor.matmul(psh[1][:], shW[:, jc, 128:256], at[:],
                                         start=(jc == 0), stop=(jc == JC - 1))
                    if with_deg:
                        nc.tensor.matmul(
                            pdeg[:], shW[:, jc, OUT_FEAT:OUT_FEAT + 1], at[:],
                            start=(jc == 0), stop=(jc == JC - 1))
                return pacc, psh, pdeg, work

            with ExitStack() as ph1:
                paccA, psh, _, workA = do_round(ph1, ROUND_A, True, False)
                for h in ROUND_A:
                    _head_epilogue(nc, h, paccA[h], psh[h // 4][
                        (h * D) % 128:(h * D) % 128 + D, :], a2rep, scol,
                        numT, zrow6, workA)
                # save M3 to SBUF before psum banks are released
                nc.scalar.copy(m3s[:, 0, :], psh[0][:])
                nc.scalar.copy(m3s[:, 1, :], psh[1][:])
            with ExitStack() as ph1b:
                paccB, _, pdeg, workB = do_round(ph1b, ROUND_B, False, True)
                for h in ROUND_B:
                    _head_epilogue(nc, h, paccB[h], m3s[
                        (h * D) % 128:(h * D) % 128 + D, h // 4, :], a2rep,
                        scol, numT, zrow6, workB)
                nc.scalar.copy(degs[:], pdeg[:])
                # finalize Z rows: z = zpart + N - deg, packed to zAll32
                for h in range(H):
                    ztmp = workB.tile([1, R], FP, name="ztmp")
                    nc.vector.scalar_tensor_tensor(
                        ztmp[:], zrow6[0:1, h, :], float(N), degs[:],
                        mybir.AluOpType.add, mybir.AluOpType.subtract)
                    nc.sync.dma_start(out=zAll32[h:h + 1, :], in_=ztmp[:])

            nc.vector.reciprocal(rzAll[:], zAll32[:])

            with ExitStack() as ph2:
                ps2 = ph2.enter_context(
                    tc.tile_pool(name="ps2", bufs=2, space="PSUM"))
                for ch in range(2):
                    pz = ps2.tile([128, R], FP, space="PSUM")
                    nc.tensor.matmul(pz[:], sel32[:, ch * 128:(ch + 1) * 128],
                                     rzAll[0:H, :], start=True, stop=True)
                    nc.vector.tensor_mul(
                        outTs[:, ch, :], numT[:, ch, :], pz[:])
                nc.sync.dma_start(out=outT[0:128, :], in_=outTs[:, 0, :])
                nc.sync.dma_start(out=outT[128:256, :], in_=outTs[:, 1, :])

    nc.compile()
    return nc


def _runs(ks):
    """Contiguous runs [k0, k1) in a sorted index list."""
    out = []
    for k in ks:
        if out and out[-1][1] == k:
            out[-1][1] = k + 1
        else:
            out.append([k, k + 1])
    return [tuple(x) for x in out]


def _head_epilogue(nc, h, pacc, m3, a2rep, scol, numT, zrow6, work):
    """numT_h = S[c] + A2[i]*(u'@B2NF) - M3[c,i]; zrow6_h = A2*(u'@B2)."""
    r0 = (h * D) % 128
    ch = h // 4
    t1 = work.tile([128, R], mybir.dt.float32, name="t1")
    nc.vector.tensor_mul(t1[r0:r0 + D, :], pacc[0:D, :],
                         a2rep[r0:r0 + D, h, :])
    nc.vector.scalar_tensor_tensor(
        numT[r0:r0 + D, ch, :], t1[r0:r0 + D, :],
        scol[r0:r0 + D, ch:ch + 1], m3,
        mybir.AluOpType.add, mybir.AluOpType.subtract)
    nc.vector.tensor_mul(zrow6[0:1, h, :], pacc[D:D + 1, :],
                         a2rep[32:33, h, :])


_PROGRAM_CACHE = {}


def kernel(x, W, b, a, adj_matrix):
    x = np.asarray(x, dtype=np.float32)
    W = np.asarray(W, dtype=np.float32)
    b = np.asarray(b, dtype=np.float32)
    a = np.asarray(a, dtype=np.float32)
    adj = np.asarray(adj_matrix, dtype=np.float32)

    xTa = np.ascontiguousarray(
        np.vstack([x.T, np.ones((1, N), np.float32)]))            # [257, N]
    wTa = np.ascontiguousarray(np.vstack([W.T, b[None, :]]))      # [257, 256]
    Ap = np.zeros((OUT_FEAT, H), np.float32)
    Ac = np.zeros((OUT_FEAT, H), np.float32)
    for h in range(H):
        Ap[h * D:(h + 1) * D, h] = a[h, :D]
        Ac[h * D:(h + 1) * D, h] = a[h, D:]
    WAp = np.ascontiguousarray(wTa @ Ap)
    WAc = np.ascontiguousarray(wTa @ Ac)
    wTa_big = np.ascontiguousarray(np.hstack([wTa, WAc]))  # [257, 264]

    sel8_host = np.zeros((H, H * 128), np.float32)
    for h in range(H):
        sel8_host[h, h * 128:(h + 1) * 128] = 1.0
    sel32_host = np.zeros((H, 256), np.float32)
    for ch in range(2):
        for m in range(128):
            sel32_host[m // 32 + 4 * ch, 128 * ch + m] = 1.0

    if "nc" not in _PROGRAM_CACHE:
        _PROGRAM_CACHE["nc"] = build_program()
    nc = _PROGRAM_CACHE["nc"]

    in_maps = []
    for c in range(N_CORES):
        rows = slice(c * R, (c + 1) * R)
        in_maps.append({
            "xTa": xTa,
            "xTrows": np.ascontiguousarray(xTa[:, rows]),
            "wTa": wTa_big,
            "WAc": WAc,
            "WAp": WAp,
            "adjT": np.ascontiguousarray(adj[rows, :].T).astype(
                ml_dtypes.bfloat16),
            "sel8": sel8_host,
            "sel32": sel32_host,
        })

    res = run_bass_kernel_spmd(nc, in_maps, list(range(N_CORES)))
    out = np.empty((N, OUT_FEAT), np.float32)
    for c in range(N_CORES):
        out[c * R:(c + 1) * R, :] = res.results[c]["outT"].T
    return out



# revision 29
# speedup vs baseline: 1.6766x; 1.6766x over previous
"""GAT layer (nn_GATLayer) Trainium2 Bass kernel.

Reference math:
    NF = x @ W.T + b                     # [N, 256] -> heads [N, 8, 32]
    lp[i,h] = sum_d NF[i,h,d] * a[h,d];  lc[j,h] = sum_d NF[j,h,d] * a[h,32+d]
    logits[i,j,h] = leaky_relu(lp+lc, 0.2) masked to 0 where adj==0
    out[i,h,:] = softmax_j(logits) @ NF[:,h,:]

Identities (adj in {0,1}):
    exp(leaky_relu(z, .2)) = exp(.2 z) * max(exp(.8 z), 1)
    u'[j,i,h] = adj[j,i] * max(A8[i,h]*B8[j,h], 1)   # one masked stream/head
    num[i,h,c] = S[c] - M3[i,c] + A2[i,h] * (u' @ (B2*NF)_h)[i,c]
    Z[i,h]     = N - deg[i]     + A2[i,h] * (u' @ B2_h)[i]
    out = num / Z
    with A8=exp(.8 lp), B8=exp(.8 lc), A2=exp(.2 lp), B2=exp(.2 lc),
    S = colsum(NF), M3 = adj @ NF, deg = adj @ 1.

Layout: 388-column "bank" layout, 4 banks of 97 cols each; bank b holds
heads (2b, 2b+1) at col offsets 0 and 64 (33 cols each: 32 feats + z/one
col), cols 33..63 zero. PSUM: 4 pacc banks [97,512] (u'@B2NF streams at
partition offsets 0/64) + 4 psh banks [97,512] (M3 + deg via ones-cols,
aligned with pacc) = 8 banks, single pass over j (32 chunks of 128).
Heads 6,7 use the rr-route: ACT computes rr=relu(A8*B8-1); the missing
"+1" rides as a 13th PE stream (stationary = aggW bank3, moving = adjT)
accumulating adj@[B2NF|B2] into pacc bank3.
"""

import numpy as np
import ml_dtypes

import concourse.bass as bass
import concourse.bacc as bacc
import concourse.tile as tile
from concourse import mybir
from concourse.bass_utils import run_bass_kernel_spmd
from concourse.masks import make_identity

N_CORES = 8
N = 4096
IN_FEAT = 256
OUT_FEAT = 256
H = 8
D = 32
R = N // N_CORES          # rows (parents) per core = 512
JC = N // 128             # j-chunks of 128 = 32
WK = OUT_FEAT + H         # NF cols + lc cols in the dense weight = 264
BANKW = 97                # per-bank col width in the 388 layout
LAYW = 4 * BANKW          # 388

FP = mybir.dt.float32
FR = mybir.dt.float32r
BF = mybir.dt.bfloat16
AF = mybir.ActivationFunctionType
ALU = mybir.AluOpType


def _col97(h):
    return BANKW * (h // 2) + 64 * (h % 2)


def build_program():
    nc = bacc.Bacc("TRN2", target_bir_lowering=False, debug=False,
                   num_devices=N_CORES)

    # host-packed inputs
    xk_in = nc.dram_tensor("xk", [128, JC * 256], FR, kind="ExternalInput").ap()
    xr_in = nc.dram_tensor("xr", [128, 4 * 256 + 2], FP,
                           kind="ExternalInput").ap()
    wk_in = nc.dram_tensor("wkk", [128, 2 * WK], FR, kind="ExternalInput").ap()
    wk2_in = nc.dram_tensor("wk2", [1, WK + 129], FR, kind="ExternalInput").ap()
    wap_in = nc.dram_tensor("wap", [128, 2 * H], FP, kind="ExternalInput").ap()
    wap2_in = nc.dram_tensor("wap2", [1, H], FP, kind="ExternalInput").ap()
    adj_in = nc.dram_tensor("adjc", [128, JC * R], BF, kind="ExternalInput").ap()
    s97_in = nc.dram_tensor("sel97h", [2, BANKW], FR, kind="ExternalInput").ap()
    sZ_in = nc.dram_tensor("selZh", [BANKW, 2], FP, kind="ExternalInput").ap()
    wsb_in = nc.dram_tensor("wsb", [128, 2 * OUT_FEAT], BF,
                            kind="ExternalInput").ap()
    ws2_in = nc.dram_tensor("ws2", [1, OUT_FEAT], BF, kind="ExternalInput").ap()
    xsb_in = nc.dram_tensor("xsb", [128, 2], BF, kind="ExternalInput").ap()
    outB = nc.dram_tensor("outB", [LAYW, R], FP, kind="ExternalOutput").ap()

    with tile.TileContext(nc) as tc:
        from contextlib import ExitStack
        with ExitStack() as top:
            consts = top.enter_context(tc.tile_pool(name="consts", bufs=1))
            persist = top.enter_context(tc.tile_pool(name="persist", bufs=1))
            atpool = top.enter_context(tc.tile_pool(name="at", bufs=3))

            ident = consts.tile([128, 128], FP)
            make_identity(nc, ident[:])
            wk2full = consts.tile([1, WK + 129], FR)
            negone = consts.tile([128, 1], FP)
            nc.vector.memset(negone[:], -1.0)
            sel97 = consts.tile([2, BANKW], FR)
            selZ = consts.tile([BANKW, 2], FP)

            wkk = consts.tile([128, 2, WK], FR)
            wap = consts.tile([128, 2, H], FP)
            wsb = consts.tile([128, 2, OUT_FEAT], BF)
            ws2 = consts.tile([1, OUT_FEAT], BF)
            xsb = consts.tile([128, 2], BF)
            nb1 = consts.tile([1, 1], BF)
            nc.vector.memset(nb1[:], float(N))
            onesrowF = consts.tile([1, 128], FP)
            nc.vector.memset(onesrowF[:], 1.0)
            nc.sync.dma_start(out=wap.rearrange("p a b -> p (a b)"),
                              in_=wap_in[:])
            wap2 = consts.tile([1, H], FP)

            nc.sync.dma_start(out=wap2[:], in_=wap2_in[:])

            # persistent SBUF
            shW = persist.tile([128, JC, LAYW], BF)
            aggW = persist.tile([128, JC, LAYW], BF)
            b8c = persist.tile([128, JC, H], FP)
            b2t = persist.tile([128, JC, H], BF)
            a8rep = persist.tile([128, H, R], BF)
            a2rep = persist.tile([128, H, R], FP)
            lpT = persist.tile([H, R], FP)
            a8Tb = persist.tile([H, R], BF)
            a2T = persist.tile([H, R], FP)
            a8st = persist.tile([1, H * R], BF)
            a2st = persist.tile([1, H * R], FP)
            scol = persist.tile([128, 4], FP)
            numT = persist.tile([128, 4, R], FP)
            t1s = persist.tile([128, 4, R], FP)
            tmpP = persist.tile([BANKW, 4, R], FP)
            pzS = persist.tile([BANKW, 4, R], FP)
            outTs = persist.tile([128, 4, R], FP)
            rzv = persist.tile([2, 4, R], FR)

            # shW gaps must be finite: psh gap partitions feed numT's gap
            # rows which the one-hot Z-extraction matmul contracts over
            # (0 * NaN = NaN). aggW gaps only feed pacc partitions 33-63,
            # which are never read, so they stay uninitialized.
            shw_gap = bass.AP(tensor=shW.tensor, offset=shW.offset + 33,
                              ap=[shW.ap[0], [LAYW, JC], [BANKW, 4], [1, 31]])
            nc.vector.memset(shw_gap, 0.0)
            shw_one = bass.AP(tensor=shW.tensor, offset=shW.offset + 32,
                              ap=[shW.ap[0], [LAYW, JC], [BANKW, 4], [64, 2]])
            nc.vector.memset(shw_one, 1.0)
            nc.vector.memset(t1s[32:64, :, :], 0.0)
            nc.vector.memset(t1s[96:128, :, :], 0.0)

            # ---- Phase 0 ----
            with ExitStack() as ph0:
                xpool = ph0.enter_context(tc.tile_pool(name="xp", bufs=1))
                ps0 = ph0.enter_context(
                    tc.tile_pool(name="ps0", bufs=1, space="PSUM"))
                psnf = ph0.enter_context(
                    tc.tile_pool(name="psnf", bufs=3, space="PSUM"))

                xr = xpool.tile([128, 4, 2, 128], FP)
                xrf = xr.rearrange("p a b c -> p (a b c)")
                nc.sync.dma_start(out=xrf[:], in_=xr_in[:, 0:1024])
                nc.sync.dma_start(out=wsb.rearrange("p a b -> p (a b)"),
                                  in_=wsb_in[:])
                nc.sync.dma_start(out=ws2[:], in_=ws2_in[:])
                nc.sync.dma_start(out=xsb[:], in_=xsb_in[:])
                nc.sync.dma_start(out=wkk.rearrange("p a b -> p (a b)"),
                                  in_=wk_in[:])
                nc.sync.dma_start(out=wk2full[:], in_=wk2_in[:])
                wk2v = wk2full[0:1, 0:WK]
                onesrow = wk2full[0:1, WK:WK + 128]
                none1 = wk2full[0:1, WK + 128:WK + 129]
                nc.sync.dma_start(out=sel97[:], in_=s97_in[:])
                nc.sync.dma_start(out=selZ[:], in_=sZ_in[:])
                xk = xpool.tile([128, JC, 2, 128], FR)
                xkf = xk.rearrange("p a b c -> p (a b c)")
                for q in range(4):
                    nc.sync.dma_start(out=xkf[:, q * 2048:(q + 1) * 2048],
                                      in_=xk_in[:, q * 2048:(q + 1) * 2048])

                # prefetch first two adjacency blocks during phase 0
                at_tiles = {}
                for blk in (0, 1):
                    t = atpool.tile([128, 4, R], BF, name="at4")
                    nc.sync.dma_start(
                        out=t.rearrange("p a b -> p (a b)"),
                        in_=adj_in[:, blk * 4 * R:(blk + 1) * 4 * R])
                    at_tiles[blk] = t

                # lp chain for own rows
                for rb in range(4):
                    plp = ps0.tile([128, H], FP, space="PSUM", tag="plp")
                    nc.tensor.matmul(plp[:], xr[:, rb, 0, :], wap[:, 0, :],
                                     start=True, stop=False)
                    nc.tensor.matmul(plp[:], xr[:, rb, 1, :], wap[:, 1, :],
                                     start=False, stop=False)
                    nc.tensor.matmul(plp[:], onesrowF[:], wap2[:],
                                     start=False, stop=True)
                    lps = xpool.tile([128, H], FP, name="lps")
                    nc.scalar.copy(lps[:], plp[:])
                    plpT = ps0.tile([H, 128], FP, space="PSUM", tag="plp")
                    nc.tensor.transpose(plpT[:], lps[:], ident[:])
                    nc.scalar.copy(lpT[:, rb * 128:(rb + 1) * 128], plpT[:])

                nc.scalar.activation(a8Tb[:], lpT[:], AF.Exp, bias=0.0,
                                     scale=0.8)
                nc.scalar.activation(a2T[:], lpT[:], AF.Exp, bias=0.0,
                                     scale=0.2)
                # collapse the 8 rows into one partition line, then
                # partition_broadcast (Pool, SBUF-only) per head
                nc.sync.dma_start(out=a8st[:], in_=a8Tb[:])
                nc.sync.dma_start(out=a2st[:], in_=a2T[:])

                def rep_pair(k):
                    h = k % H
                    if k < H:
                        nc.gpsimd.partition_broadcast(
                            a8rep[:, h, :], a8st[0:1, h * R:(h + 1) * R])
                    else:
                        nc.gpsimd.partition_broadcast(
                            a2rep[:, h, :], a2st[0:1, h * R:(h + 1) * R])

                # S columns: scolP[p, b] over 3 k-chunks, rows 0-31 & 64-95
                scolP = ps0.tile([128, 4], FP, space="PSUM", tag="plp")
                for b in range(4):
                    for e in range(2):
                        cols = slice(64 * b + 32 * e, 64 * b + 32 * e + 32)
                        out_ap = scolP[64 * e:64 * e + 32, b:b + 1]
                        nc.tensor.matmul(out_ap, wsb[:, 0, cols],
                                         xsb[:, 0:1], start=True, stop=False)
                        nc.tensor.matmul(out_ap, wsb[:, 1, cols],
                                         xsb[:, 1:2], start=False, stop=False)
                        nc.tensor.matmul(out_ap, ws2[0:1, cols], nb1[:],
                                         start=False, stop=True)
                nc.vector.memset(scolP[32:64, :], float(N))
                nc.vector.memset(scolP[96:128, :], float(N))
                nc.scalar.copy(scol[:], scolP[:])

                # aggW sweep: aggW[:, nb, 33-blocks] = shW * b2
                def agg_sweep(nb, eng):
                    sv = bass.AP(tensor=shW.tensor,
                                 offset=shW.offset + nb * LAYW,
                                 ap=[shW.ap[0], [BANKW, 4], [64, 2], [1, 33]])
                    av = bass.AP(tensor=aggW.tensor,
                                 offset=aggW.offset + nb * LAYW,
                                 ap=[aggW.ap[0], [BANKW, 4], [64, 2], [1, 33]])
                    bv = bass.AP(tensor=b2t.tensor, offset=b2t.offset + nb * H,
                                 ap=[b2t.ap[0], [2, 4], [1, 2], [0, 33]])
                    eng.tensor_mul(av, sv, bv)

                # NF loop: pairs of chunks; pnf2 = [128, 2, 512] (2 banks)
                PA_ENG = [nc.vector, nc.gpsimd, nc.scalar, nc.vector,
                          nc.gpsimd, nc.scalar, nc.vector, nc.gpsimd]
                for t in range(JC // 2):
                    pnf2 = psnf.tile([128, 2, 512], FP, space="PSUM",
                                     tag="pnf2")
                    for e in range(2):
                        nb = 2 * t + e
                        pnf = pnf2[:, e, 0:WK]
                        nc.tensor.matmul(pnf, xk[:, nb, 0, :], wkk[:, 0, :],
                                         start=True, stop=False)
                        nc.tensor.matmul(pnf, xk[:, nb, 1, :], wkk[:, 1, :],
                                         start=False, stop=False)
                        nc.tensor.matmul(pnf, onesrow[:], wk2v[:],
                                         start=False, stop=True)
                    for e in range(2):
                        nb = 2 * t + e
                        dst = bass.AP(
                            tensor=shW.tensor,
                            offset=shW.offset + nb * LAYW,
                            ap=[shW.ap[0], [BANKW, 4], [64, 2], [1, 32]])
                        srcv = bass.AP(
                            tensor=pnf2.tensor,
                            offset=pnf2.offset + e * 512,
                            ap=[pnf2.ap[0], [64, 4], [32, 2], [1, 32]])
                        if nb % 8 in (3, 6, 7):
                            nc.scalar.copy(dst, srcv)
                        else:
                            nc.vector.tensor_copy(dst, srcv)
                    lcsrc = bass.AP(tensor=pnf2.tensor,
                                    offset=pnf2.offset + 256,
                                    ap=[pnf2.ap[0], [512, 2], [1, H]])
                    nb0 = 2 * t
                    nc.scalar.activation(
                        b8c[:, nb0:nb0 + 2, :].rearrange("p a b -> p (a b)"),
                        lcsrc, AF.Exp, bias=0.0, scale=0.8)
                    nc.scalar.activation(
                        b2t[:, nb0:nb0 + 2, :].rearrange("p a b -> p (a b)"),
                        lcsrc, AF.Exp, bias=0.0, scale=0.2)
                    if nb0 < 19:
                        agg_sweep(nb0, nc.vector if nb0 % 3 != 2
                                  else nc.gpsimd)
                    if nb0 + 1 < 19:
                        agg_sweep(nb0 + 1, nc.vector if (nb0 + 1) % 3 != 2
                                  else nc.gpsimd)
                    rep_pair(t)

            # ---- Phase 1: single-pass j-loop ----
            with ExitStack() as ph1:
                acc = ph1.enter_context(
                    tc.tile_pool(name="acc", bufs=1, space="PSUM"))
                work = ph1.enter_context(tc.tile_pool(name="work", bufs=2))
                pacc = [acc.tile([BANKW, R], FP, space="PSUM",
                                 name=f"pacc{b}") for b in range(4)]
                psh = [acc.tile([BANKW, R], FP, space="PSUM",
                                name=f"psh{b}") for b in range(4)]
                for jc in range(JC):
                    blk = jc // 4
                    if jc % 4 == 0 and blk + 2 < JC // 4:
                        t = atpool.tile([128, 4, R], BF, name="at4")
                        nc.sync.dma_start(
                            out=t.rearrange("p a b -> p (a b)"),
                            in_=adj_in[:, (blk + 2) * 4 * R:(blk + 3) * 4 * R])
                        at_tiles[blk + 2] = t
                    at4 = at_tiles[blk]
                    at = at4[:, jc % 4, :]
                    tb = work.tile([128, H, R], BF, name="tb")
                    # tb h0,h1 on DVE; h2-4 on Pool; h5 chain + h6,h7 rr on ACT
                    for h in (0, 1):
                        nc.vector.tensor_scalar(tb[:, h, :], a8rep[:, h, :],
                                                b8c[:, jc, h:h + 1], 1.0,
                                                ALU.mult, ALU.max)
                    for h in (2, 3, 4):
                        nc.gpsimd.tensor_scalar(tb[:, h, :], a8rep[:, h, :],
                                                b8c[:, jc, h:h + 1], 1.0,
                                                ALU.mult, ALU.max)
                    rr5 = work.tile([128, R], BF, name="rr5")
                    nc.scalar.activation(rr5[:], a8rep[:, 5, :], AF.Relu,
                                         bias=negone[:],
                                         scale=b8c[:, jc, 5:6])
                    nc.scalar.activation(tb[:, 5, :], rr5[:], AF.Copy,
                                         bias=1.0, scale=1.0)
                    for h in (6, 7):
                        nc.scalar.activation(tb[:, h, :], a8rep[:, h, :],
                                             AF.Relu, bias=negone[:],
                                             scale=b8c[:, jc, h:h + 1])
                    # merged mask: s1 = tb * at (broadcast over head dim)
                    s1 = work.tile([128, H, R], BF, name="s1")
                    atb = bass.AP(tensor=at.tensor, offset=at.offset,
                                  ap=[at.ap[0], [0, H], at.ap[1]])
                    nc.vector.tensor_mul(s1[:], tb[:], atb)
                    if jc % 2 == 0 and 19 + jc // 2 < JC:
                        agg_sweep(19 + jc // 2, nc.vector)
                    # PE streams: 4 psh + aggX(bank3) + 8 s1
                    st = (jc == 0)
                    sp = (jc == JC - 1)
                    for b in range(4):
                        nc.tensor.matmul(
                            psh[b][:], shW[:, jc, b * BANKW:(b + 1) * BANKW],
                            at[:], start=st, stop=sp, skip_group_check=True)
                    if not sp:
                        nc.tensor.matmul(
                            pacc[3][:], aggW[:, jc, 3 * BANKW:4 * BANKW],
                            at[:], start=st, stop=False,
                            skip_group_check=True)
                    for h in range(H):
                        b, off = h // 2, 64 * (h % 2)
                        rrh = h >= 6
                        nc.tensor.matmul(
                            pacc[b][off:off + 33, :],
                            aggW[:, jc, _col97(h):_col97(h) + 33],
                            s1[:, h, :],
                            start=(st and not rrh),
                            stop=(sp and not rrh),
                            skip_group_check=True)
                    if sp:
                        nc.tensor.matmul(
                            pacc[3][:], aggW[:, jc, 3 * BANKW:4 * BANKW],
                            at[:], start=False, stop=True,
                            skip_group_check=True)

                # epilogue per bank, fully pipelined: t1 = pacc*a2;
                # numT = (t1+scol)-psh; rz = 1/Z; pz = bcast(rz) reusing the
                # psh bank; out = numT*pz; DMA out.
                pstride = numT.ap[0][0]
                with nc.allow_low_precision(reason="1/Z in f32r"):
                    for b in range(4):
                        # odd-head slice staged via Act, multiplied on Pool
                        nc.scalar.copy(tmpP[64:97, b, :], pacc[b][64:97, :])
                        nc.gpsimd.tensor_mul(t1s[64:97, b, :],
                                             tmpP[64:97, b, :],
                                             a2rep[64:97, 2 * b + 1, :])
                        nc.vector.tensor_mul(t1s[0:33, b, :],
                                             pacc[b][0:33, :],
                                             a2rep[0:33, 2 * b, :])
                        nc.vector.scalar_tensor_tensor(
                            numT[0:BANKW, b, :], t1s[0:BANKW, b, :],
                            scol[0:BANKW, b:b + 1], psh[b][:],
                            ALU.add, ALU.subtract)
                        nc.tensor.matmul(psh[b][0:2, :], selZ[:],
                                         numT[0:BANKW, b, :],
                                         start=True, stop=True,
                                         skip_group_check=True)
                        nc.vector.reciprocal(rzv[:, b, :], psh[b][0:2, :])
                        nc.tensor.matmul(psh[b][:], sel97[:], rzv[:, b, :],
                                         start=True, stop=True,
                                         skip_group_check=True)
                        nc.scalar.copy(pzS[0:BANKW, b, :], psh[b][:])
                        nc.gpsimd.tensor_mul(outTs[0:BANKW, b, :],
                                             numT[0:BANKW, b, :],
                                             pzS[0:BANKW, b, :])
                        nc.sync.dma_start(
                            out=outB[b * BANKW:(b + 1) * BANKW, :],
                            in_=outTs[0:BANKW, b, :])

    nc.compile()
    return nc


_PROGRAM_CACHE = {}


def kernel(x, W, b, a, adj_matrix):
    x = np.asarray(x, dtype=np.float32)
    W = np.asarray(W, dtype=np.float32)
    b = np.asarray(b, dtype=np.float32)
    a = np.asarray(a, dtype=np.float32)
    adj = np.asarray(adj_matrix, dtype=np.float32)

    wTa = np.vstack([W.T, b[None, :]])                      # [257, 256]
    Ap = np.zeros((OUT_FEAT, H), np.float32)
    Ac = np.zeros((OUT_FEAT, H), np.float32)
    for h in range(H):
        Ap[h * D:(h + 1) * D, h] = a[h, :D]
        Ac[h * D:(h + 1) * D, h] = a[h, D:]
    WAp = wTa @ Ap                                          # [257, 8]
    WAc = wTa @ Ac
    wk_full = np.hstack([wTa, WAc])                         # [257, 264]
    wkk_host = np.ascontiguousarray(
        wk_full[0:256].reshape(2, 128, WK).transpose(1, 0, 2).reshape(128, -1))
    wsb_host = np.ascontiguousarray(
        wk_full[0:256, 0:256].reshape(2, 128, 256).transpose(1, 0, 2)
        .reshape(128, -1)).astype(ml_dtypes.bfloat16)
    ws2_host = wk_full[256:257, 0:256].astype(ml_dtypes.bfloat16)
    wk2_host = np.empty((1, WK + 129), np.float32)
    wk2_host[0, 0:WK] = wk_full[256]
    wk2_host[0, WK:WK + 128] = 1.0
    wk2_host[0, WK + 128] = float(N)
    wap_host = np.ascontiguousarray(
        WAp[0:256].reshape(2, 128, H).transpose(1, 0, 2).reshape(128, -1))
    wap2_host = np.ascontiguousarray(WAp[256:257])

    # xk[p, jc*256 + k*128 + c] = x[jc*128+c, k*128+p]
    xk_host = np.ascontiguousarray(
        x.reshape(JC, 128, 2, 128).transpose(3, 0, 2, 1).reshape(128, -1))
    xsum = x.sum(axis=0, dtype=np.float64).astype(np.float32)  # [256]
    xsb_host = np.empty((128, 2), np.float32)
    xsb_host[:, 0] = xsum[0:128]
    xsb_host[:, 1] = xsum[128:256]
    xsb_host = xsb_host.astype(ml_dtypes.bfloat16)
    sel97_host = np.zeros((2, BANKW), np.float32)
    sel97_host[0, 0:33] = 1.0
    sel97_host[1, 64:97] = 1.0
    selZ_host = np.zeros((BANKW, 2), np.float32)
    selZ_host[32, 0] = 1.0
    selZ_host[96, 1] = 1.0

    if "nc" not in _PROGRAM_CACHE:
        _PROGRAM_CACHE["nc"] = build_program()
    nc = _PROGRAM_CACHE["nc"]

    in_maps = []
    for c in range(N_CORES):
        rows = slice(c * R, (c + 1) * R)
        xrow = x[rows]                                       # [512, 256]
        xr_host = np.empty((128, 4 * 256 + 2), np.float32)
        xr_host[:, 0:1024] = (
            xrow.reshape(4, 128, 2, 128).transpose(3, 0, 2, 1).reshape(128, -1))
        xr_host[:, 1024] = xsum[0:128]
        xr_host[:, 1025] = xsum[128:256]
        adjc_host = np.ascontiguousarray(
            adj[rows, :].T.reshape(JC, 128, R).transpose(1, 0, 2)
            .reshape(128, -1)).astype(ml_dtypes.bfloat16)
        in_maps.append({
            "xk": xk_host,
            "xr": xr_host,
            "wkk": wkk_host,
            "wk2": wk2_host,
            "wap": wap_host,
            "wap2": wap2_host,
            "adjc": adjc_host,
            "sel97h": sel97_host,
            "selZh": selZ_host,
            "wsb": wsb_host,
            "ws2": ws2_host,
            "xsb": xsb_host,
        })

    res = run_bass_kernel_spmd(nc, in_maps, list(range(N_CORES)))
    out = np.empty((N, OUT_FEAT), np.float32)
    for c in range(N_CORES):
        ob = res.results[c]["outB"]                          # [388, 512]
        for h in range(H):
            c0 = _col97(h)
            out[c * R:(c + 1) * R, h * D:(h + 1) * D] = ob[c0:c0 + 32].T
    return out
